# revision 39
# baseline (speedup 1.0000x reference)
"""Trainium2 Bass kernel for nn_AdapterAttnForMamba (depthwise 3x3 conv over a
pad-token-augmented 66x66 image + spatial-transpose permutation + residual).

Math (per batch b, channel c):
  out(i,j) = x(i,j) + y(j,i) + bias_c
  y(r,s)   = sum_{a,b in 0..2} w[c,a,b] * V[r+a-1, s+b-1]
  V        = 65x65 "virtual" image: V[r<64, s<64] = x(r,s); V[r<64, 64] = tok0;
             V[64, s] = tok_{s%2}; zero outside (SAME conv padding).

Strategy: pure data parallel over batch (2 batches / core, 8 cores).
Channels-on-partitions layout via TensorE transposes.  The conv runs in
fp8e4 with MatmulPerfMode.DoubleRow: the 9 taps are packed into 5 DR
matmuls per 8-row chunk (each DR matmul computes two taps at once; the
odd 9th tap rides with a zero-weight dummy partner).  DR requires the
pair-dim AP step to be a multiple of 16 bytes and the rhs base offset to
be 4-byte aligned, while tap column offsets span {0,1,2} bytes — so the
padded image is kept in THREE byte-shifted copies Q0/Q1/Q2 (Qv[k] =
Q0[k+v]) packed in one SBUF tile at 16-aligned gaps of QSZ.  Q0 is
written once (ACT evac of TensorE transposes + pad/ring writes); Q1/Q2
are SBUF->SBUF DMA copies.

Pipeline per (batch bi, channel-block ct of 128), software-pipelined one
ct ahead so phase B never waits on the Q copies:
  x stripes [128pix,1024ch] --(SWDGE cast f32->bf16)--> SBUF
  TensorE transpose -> PSUM [128ch,128pix] -> ACT copies into Q0
  rings/pads into Q0 (DVE), Q1/Q2 shifted copies (DMA)
  5 fp8 DoubleRow matmuls per 8-row chunk accumulate conv into PSUM
  (weights pre-scaled x32 so fp8 stays in normal range)
  ACT/DVE evac + 32*conv_b bias -> u[ct] (bf16, natural y(r,s) order)
  TensorE transpose back with permuted lhsT AP and (1/32)*I moving
  DVE adds residual x stripe; DMA f32 to DRAM out.
"""

import os
import sys

import numpy as np

for _p in ("/opt/trn_rl_repo", "/root/.axon_site/_ro/trn_rl_repo"):
    if os.path.isdir(_p) and _p not in sys.path:
        sys.path.append(_p)

B, H, W, C = 16, 64, 64, 1024
L = H * W  # 4096
NCORES = 8
BPC = B // NCORES  # batches per core
NCT = C // 128  # channel blocks
RS = 72  # Q row stride (bytes == fp8 elements); must be 0 mod 8 so that
# pair steps 2*RS and QSZ are 16-aligned and row bases are 4-aligned
PROWS = 66  # Q rows: image rows -1..64 (+1 ring)
QSZ = PROWS * RS  # 5280, 16-aligned
NSTRIPE = L // 128  # 32 pixel stripes per batch

_CACHE = {}


def _expected_seq_idx():
    return np.arange(L).reshape(H, W).T.reshape(-1)


def _build_nc():
    import concourse.mybir as mybir
    from concourse import bacc
    from concourse.masks import make_identity
    from concourse.tile import TileContext

    from concourse.ap import AP

    f32 = mybir.dt.float32
    bf16 = mybir.dt.bfloat16
    fp8 = mybir.dt.float8e4
    DR = mybir.MatmulPerfMode.DoubleRow
    Copy = mybir.ActivationFunctionType.Copy
    Ident = mybir.ActivationFunctionType.Identity

    nc = bacc.Bacc(None, target_bir_lowering=False)
    x_ext = nc.declare_dram_parameter("x", [BPC, L, C], f32, isOutput=False)
    tok_ext = nc.declare_dram_parameter("pad_token", [1, C, 2], f32, isOutput=False)
    w_ext = nc.declare_dram_parameter("conv_w", [C, 1, 3, 3], f32, isOutput=False)
    b_ext = nc.declare_dram_parameter("conv_b", [C], f32, isOutput=False)
    out_ext = nc.declare_dram_parameter("out", [BPC, L, C], f32, isOutput=True)

    with TileContext(nc) as tc:
        with (
            tc.tile_pool(name="const", bufs=1) as constp,
            tc.tile_pool(name="xpool", bufs=54) as xpool,
            tc.tile_pool(name="zpool", bufs=8) as zpool,
            tc.tile_pool(name="qpool", bufs=3) as qpool,
            tc.tile_pool(name="opool", bufs=3) as opool,
            tc.tile_pool(name="ps_t", bufs=2, space="PSUM") as ps_t,
            tc.tile_pool(name="ps_z", bufs=2, space="PSUM") as ps_z,
            tc.tile_pool(name="ps_o", bufs=2, space="PSUM") as ps_o,
        ):
            # ---- constants ----
            ident = constp.tile([128, 128], bf16, tag="ident")
            make_identity(nc, ident)
            zeros = constp.tile([128, 128], bf16, tag="zeros")
            nc.vector.memset(zeros[:], 0.0)
            ident32 = constp.tile([128, 128], bf16, tag="ident32")
            nc.vector.tensor_scalar_mul(out=ident32[:], in0=ident[:], scalar1=32.0)
            identInv = constp.tile([128, 128], bf16, tag="identInv")
            nc.vector.tensor_scalar_mul(out=identInv[:], in0=ident[:], scalar1=1.0 / 32.0)

            wt = constp.tile([128, 9 * NCT], f32, tag="wt")
            cb = constp.tile([128, NCT], f32, tag="cb")
            cb32 = constp.tile([128, NCT], f32, tag="cb32")
            tokt = constp.tile([128, 2 * NCT], f32, tag="tokt")
            nc.sync.dma_start(
                out=wt.rearrange("p (ct t) -> p ct t", t=9),
                in_=w_ext.rearrange("(ct p) a k l -> p ct (a k l)", ct=NCT),
            )
            nc.sync.dma_start(
                out=cb[:],
                in_=b_ext.rearrange("(ct p) -> p ct", ct=NCT),
            )
            nc.sync.dma_start(
                out=tokt.rearrange("p (ct two) -> p ct two", two=2),
                in_=tok_ext.rearrange("a (ct p) two -> p ct (a two)", ct=NCT),
            )
            nc.vector.tensor_scalar_mul(out=cb32[:], in0=cb[:], scalar1=32.0)

            # ---- DoubleRow stationaries: 6 per ct, [128, 2, 128] fp8 ----
            # d[k, m, i] = delta(k,i) * 32 * w_m(k).  Pairings (m0, m1):
            #   dA: (-1,-1),(+1,-1)   dB: (-1,0),(+1,0)   dC: (-1,+1),(+1,+1)
            #   dD: (0,-1),(0,0)      dE: (0,+1), zero    dF: zero, (0,+1)
            def tcol(ct, di, dj):
                return ct * 9 + (di + 1) * 3 + (dj + 1)

            PAIRS = [
                ("dA", (-1, -1), (1, -1)),
                ("dB", (-1, 0), (1, 0)),
                ("dC", (-1, 1), (1, 1)),
                ("dD", (0, -1), (0, 0)),
                ("dE", (0, 1), None),
                ("dF", None, (0, 1)),
            ]
            dstat = {}  # (ct, name) -> tile
            for ct in range(NCT):
                for name, t0, t1 in PAIRS:
                    d = constp.tile([128, 256], fp8, tag=f"{name}{ct}")
                    dv = d.rearrange("p (m c) -> p m c", m=2)
                    for m, tt in ((0, t0), (1, t1)):
                        if tt is None:
                            nc.vector.memset(dv[:, m : m + 1, :], 0.0)
                        else:
                            nc.vector.tensor_scalar_mul(
                                out=dv[:, m : m + 1, :],
                                in0=ident32[:],
                                scalar1=wt[:, tcol(ct, *tt) : tcol(ct, *tt) + 1],
                            )
                    dstat[(ct, name)] = d

            # pads: V[r<64, 64] = tok0 (strided col writes); V[64, s] =
            # tok_{s%2} for s=0..64 (Q0 row 65, via the pair view)
            def emit_pads(ct, Qv, Qm):
                nc.vector.tensor_scalar_add(
                    out=Qv[:, 0:1, 1:65, 65:66],
                    in0=zeros[:, 0:64],
                    scalar1=tokt[:, 2 * ct : 2 * ct + 1],
                )
                r65h = 65 * RS // 2  # pair index of row 65 start (RS even)
                nc.vector.tensor_scalar_add(  # even s -> odd cols 1,3..65
                    out=Qm[:, 0:1, r65h : r65h + 33, 1:2],
                    in0=zeros[:, 0:33],
                    scalar1=tokt[:, 2 * ct : 2 * ct + 1],
                )
                nc.vector.tensor_scalar_add(  # odd s -> even cols 2,4..64
                    out=Qm[:, 0:1, r65h + 1 : r65h + 33, 0:1],
                    in0=zeros[:, 0:32],
                    scalar1=tokt[:, 2 * ct + 1 : 2 * ct + 2],
                )

            # ---- per-(bi,ct) phase A: build Q0/Q1/Q2 ----
            def emitA(bi, ct, xs, QQs):
                QQ = qpool.tile([128, 3 * QSZ], fp8, tag="QQ")
                QQs[ct] = QQ
                Qv = QQ.rearrange("p (v r c) -> p v r c", v=3, c=RS)
                Qm = QQ.rearrange("p (v m x) -> p v m x", v=3, x=2)
                # ring zeros and pad tokens first (no transpose dependency),
                # so the Q1/Q2 copies only wait on the last ACT evac
                nc.vector.memset(Qv[:, 0:1, 0:1, 0:66], 0.0)
                nc.vector.memset(Qv[:, 0:1, 1:66, 0:1], 0.0)
                emit_pads(ct, Qv, Qm)
                for g in range(8):  # 4 consecutive stripes per group
                    pst = ps_t.tile([128, 512], f32, tag="pst")
                    for k in range(4):
                        s = g * 4 + k
                        nc.tensor.matmul(
                            pst[:, k * 128 : (k + 1) * 128],
                            xs[s][:, ct * 128 : (ct + 1) * 128],
                            ident[:],
                            start=True,
                            stop=True,
                        )
                    # pst[c, il*64+j] = x(8g+il, j) -> Q0 rows 8g+1..8g+9,
                    # cols 1..65 (contiguous inner runs of 64).  3 of 8
                    # copies go on DVE to balance engine load.
                    if g in (1, 3, 5):
                        nc.vector.tensor_copy(
                            out=Qv[:, 0:1, 8 * g + 1 : 8 * g + 9, 1:65],
                            in_=pst.rearrange("p (il j) -> p il j", il=8),
                        )
                    else:
                        nc.scalar.activation(
                            out=Qv[:, 0:1, 8 * g + 1 : 8 * g + 9, 1:65],
                            in_=pst.rearrange("p (il j) -> p il j", il=8),
                            func=Copy,
                            scale=1.0,
                        )
                # shifted copies: Q1[k] = Q0[k+1], Q2[k] = Q0[k+2]
                nc.scalar.dma_start(
                    out=QQ[:, QSZ : 2 * QSZ - 1], in_=QQ[:, 1:QSZ]
                )
                nc.sync.dma_start(
                    out=QQ[:, 2 * QSZ : 3 * QSZ - 2], in_=QQ[:, 2:QSZ]
                )

            # ---- per-(bi,ct) phase B: conv -> u (natural y order) ----
            # chunk n computes y(r,s) for rows r=8n..8n+7 (s-inner); the 5 DR
            # matmuls accumulate in one PSUM bank.  rhs AP dims:
            # [[3*QSZ,128],[pairD,2],[RS,8],[1,64]]; base must be 4B-aligned
            # and pairD 16B-aligned, hence the Q0/Q1/Q2 variants.
            def emitB(bi, ct, QQs, z_tiles):
                QQ = QQs[ct]
                u = zpool.tile([128, L], fp8, tag="u")
                # u[c, r*64 + s] = y(r,s) + cb, r-major: the evac stays
                # contiguous.  (Every attempt to produce s-major u cost more
                # than phase C saves: DVE scatter-out 5x, PE out-AP scatter
                # ~2x on the DR matmuls, ACT scatter is free from SBUF but
                # 4x from PSUM.)
                z_tiles[ct] = u
                for blk in range(4):
                    pz0 = ps_z.tile([128, 512], f32, tag="pz")
                    pz1 = ps_z.tile([128, 512], f32, tag="pz")
                    pzs = [pz0, pz1]
                    for t in range(5):
                        for half in range(2):
                            n = 2 * blk + half
                            if t < 3:  # dA/dB/dC: vertical pairs in Qt
                                off = t * QSZ + (8 * n) * RS
                                pd = 2 * RS
                                dn = ("dA", "dB", "dC")[t]
                            elif t == 3:  # dD: (0,-1)@Q0 with (0,0)@Q1
                                off = (8 * n + 1) * RS
                                pd = QSZ
                                dn = "dD"
                            else:  # (0,+1)@Q2 + zero-weight dummy
                                if n < 7:
                                    off = 2 * QSZ + (8 * n + 1) * RS
                                    dn = "dE"
                                else:  # dummy rows would overflow: swap roles
                                    off = 2 * QSZ + (8 * n - 1) * RS
                                    dn = "dF"
                                pd = 2 * RS
                            rhs = AP(
                                QQ.tensor,
                                off,
                                [[3 * QSZ, 128], [pd, 2], [RS, 8], [1, 64]],
                            )
                            lhsT = AP(
                                dstat[(ct, dn)].tensor,
                                0,
                                [[256, 128], [128, 2], [1, 128]],
                            )
                            nc.tensor.matmul(
                                pzs[half][:],
                                lhsT,
                                rhs,
                                start=(t == 0),
                                stop=(t == 4),
                                perf_mode=DR,
                            )
                    for half in range(2):
                        n = 2 * blk + half
                        if n in (0, 4):  # 2 of 8 on ACT to balance engines
                            nc.scalar.activation(
                                out=u[:, n * 512 : (n + 1) * 512],
                                in_=pzs[half][:],
                                func=Ident,
                                scale=1.0,
                                bias=cb32[:, ct : ct + 1],
                            )
                        else:
                            nc.vector.tensor_scalar_add(
                                out=u[:, n * 512 : (n + 1) * 512],
                                in0=pzs[half][:],
                                scalar1=cb32[:, ct : ct + 1],
                            )

            # ---- main loops ----
            # x loads: emit the next batch's first PRELOAD stripes right
            # after this batch's (the 44-slot ring has 12 spare slots), so
            # they stream during the DMA-idle A+B window instead of
            # competing with the out-stores inside phase C's window.
            PRELOAD = 22

            def load_stripe(bi, s):
                xt = xpool.tile([128, C], bf16, tag="x")
                # SWDGE dma casts f32 -> bf16; plain natural stripe
                nc.gpsimd.dma_start(
                    out=xt[:], in_=x_ext[bi, s * 128 : (s + 1) * 128, :]
                )
                return xt

            # ---- phase C: transpose back, residual, store (one stripe) ----
            # stripe s needs z[c, i*64+j] = u[c, j*64+i]/32 for i in
            # {2s, 2s+1}: two column-tiled matmuls per ct with
            # single-free-dim strided lhsT (stride 64; the weights AP
            # only admits one free dim) and (1/32)*I moving.
            def emitC(bi, s, xs, z_tiles):
                p2 = ps_o.tile([128, 1024], f32, tag="p2")
                for ct in range(NCT):
                    for il2 in range(2):
                        lhsT = AP(
                            z_tiles[ct].tensor,
                            2 * s + il2,
                            [[L, 128], [64, 64]],
                        )
                        nc.tensor.matmul(
                            p2[
                                il2 * 64 : (il2 + 1) * 64,
                                ct * 128 : (ct + 1) * 128,
                            ],
                            lhsT,
                            identInv[:],
                            start=True,
                            stop=True,
                        )
                ob = opool.tile([128, C], f32, tag="ob")
                nc.vector.tensor_add(out=ob[:], in0=p2[:], in1=xs[s][:])
                nc.sync.dma_start(
                    out=out_ext[bi, s * 128 : (s + 1) * 128, :], in_=ob[:]
                )

            # Schedule (BPC == 2): bi0's phase C window is DMA-bound (out
            # stores + bi1 x-in share HBM), so bi1's first 4 phase-A builds
            # are interleaved into it to hide their PE/ACT work; bi1's A/B
            # pipeline then resumes one-ct-ahead (B emitted before the A
            # that reuses its QQ ring slot, keeping WAR order acyclic).
            xs0 = [load_stripe(0, s) for s in range(NSTRIPE)]
            xs1 = [load_stripe(1, s) for s in range(PRELOAD)]
            QQ0, z0 = {}, {}
            for ci in range(NCT + 1):
                if ci < NCT:
                    emitA(0, ci, xs0, QQ0)
                if ci >= 1:
                    emitB(0, ci - 1, QQ0, z0)
            xs1 += [load_stripe(1, s) for s in range(PRELOAD, NSTRIPE)]
            QQ1, z1 = {}, {}
            APTS = {11: 0, 18: 1, 25: 2}  # stripe -> bi1 ct to prebuild
            for s in range(NSTRIPE):
                emitC(0, s, xs0, z0)
                if s in APTS:
                    emitA(1, APTS[s], xs1, QQ1)
            for ci in range(NCT):  # B0,A3,B1,A4,...,B4,A7,B5,B6,B7
                emitB(1, ci, QQ1, z1)
                if ci + 3 < NCT:
                    emitA(1, ci + 3, xs1, QQ1)
            for s in range(NSTRIPE):
                emitC(1, s, xs1, z1)


    nc.finalize()
    return nc


def _get_compiled():
    if "nc" not in _CACHE:
        _CACHE["nc"] = _build_nc()
    return _CACHE["nc"]


def _run(inputs, trace=False):
    from concourse.bass_utils import run_bass_kernel_spmd

    x = np.ascontiguousarray(np.asarray(inputs["x"], dtype=np.float32))
    pad_token = np.ascontiguousarray(np.asarray(inputs["pad_token"], dtype=np.float32))
    conv_w = np.ascontiguousarray(np.asarray(inputs["conv_w"], dtype=np.float32))
    conv_b = np.ascontiguousarray(np.asarray(inputs["conv_b"], dtype=np.float32))
    seq_idx = np.asarray(inputs["seq_idx"]).astype(np.int64)

    nc = _get_compiled()
    in_maps = []
    for k in range(NCORES):
        in_maps.append(
            {
                "x": x[k * BPC : (k + 1) * BPC],
                "pad_token": pad_token,
                "conv_w": conv_w,
                "conv_b": conv_b,
            }
        )
    res = run_bass_kernel_spmd(nc, in_maps, core_ids=list(range(NCORES)), trace=trace)
    out = np.concatenate([r["out"] for r in res.results], axis=0)

    # The device kernel hardcodes the reference's transpose permutation in its
    # access patterns. If the harness ever supplies a different seq_idx,
    # correct on host: out = x + y[:, seq_idx]  with y = (out_dev - x) at the
    # hardcoded permutation undone.
    exp = _expected_seq_idx()
    if not np.array_equal(seq_idx, exp):
        y = (out - x)[:, np.argsort(exp), :]
        out = x + y[:, seq_idx, :]

    return out, getattr(res, "exec_time_ns", None)


def kernel(**inputs) -> np.ndarray:
    out, _ = _run(inputs, trace=False)
    return out


# revision 40
# speedup vs baseline: 1.0411x; 1.0411x over previous
"""Trainium2 Bass kernel for nn_AdapterAttnForMamba (depthwise 3x3 conv over a
pad-token-augmented 66x66 image + spatial-transpose permutation + residual).

Math (per batch b, channel c):
  out(i,j) = x(i,j) + y(j,i) + bias_c
  y(r,s)   = sum_{a,b in 0..2} w[c,a,b] * V[r+a-1, s+b-1]
  V        = 65x65 "virtual" image: V[r<64, s<64] = x(r,s); V[r<64, 64] = tok0;
             V[64, s] = tok_{s%2}; zero outside (SAME conv padding).

Strategy: pure data parallel over batch (2 batches / core, 8 cores).
Channels-on-partitions layout via TensorE transposes.  The conv runs in
fp8e4 with MatmulPerfMode.DoubleRow: the 9 taps are packed into 5 DR
matmuls per 8-row chunk (each DR matmul computes two taps at once; the
odd 9th tap rides with a zero-weight dummy partner).  DR requires the
pair-dim AP step to be a multiple of 16 bytes and the rhs base offset to
be 4-byte aligned, while tap column offsets span {0,1,2} bytes — so the
padded image is kept in THREE byte-shifted copies Q0/Q1/Q2 (Qv[k] =
Q0[k+v]) packed in one SBUF tile at 16-aligned gaps of QSZ.  Q0 is
written once (ACT evac of TensorE transposes + pad/ring writes); Q1/Q2
are SBUF->SBUF DMA copies.

Pipeline per (batch bi, channel-block ct of 128), software-pipelined one
ct ahead so phase B never waits on the Q copies:
  x stripes [128pix,1024ch] --(SWDGE cast f32->bf16)--> SBUF
  TensorE transpose -> PSUM [128ch,128pix] -> ACT copies into Q0
  rings/pads into Q0 (DVE), Q1/Q2 shifted copies (DMA)
  5 fp8 DoubleRow matmuls per 8-row chunk accumulate conv into PSUM
  (weights pre-scaled x32 so fp8 stays in normal range)
  ACT/DVE evac + 32*conv_b bias -> u[ct] (bf16, natural y(r,s) order)
  TensorE transpose back with permuted lhsT AP and (1/32)*I moving
  DVE adds residual x stripe; DMA f32 to DRAM out.
"""

import os
import sys

import numpy as np

for _p in ("/opt/trn_rl_repo", "/root/.axon_site/_ro/trn_rl_repo"):
    if os.path.isdir(_p) and _p not in sys.path:
        sys.path.append(_p)

B, H, W, C = 16, 64, 64, 1024
L = H * W  # 4096
NCORES = 8
BPC = B // NCORES  # batches per core
NCT = C // 128  # channel blocks
RS = 72  # Q row stride (bytes == fp8 elements); must be 0 mod 8 so that
# pair steps 2*RS and QSZ are 16-aligned and row bases are 4-aligned
PROWS = 66  # Q rows: image rows -1..64 (+1 ring)
QSZ = PROWS * RS  # 5280, 16-aligned
NSTRIPE = L // 128  # 32 pixel stripes per batch

_CACHE = {}


def _expected_seq_idx():
    return np.arange(L).reshape(H, W).T.reshape(-1)


def _build_nc():
    import concourse.mybir as mybir
    from concourse import bacc
    from concourse.masks import make_identity
    from concourse.tile import TileContext

    from concourse.ap import AP

    f32 = mybir.dt.float32
    bf16 = mybir.dt.bfloat16
    fp8 = mybir.dt.float8e4
    DR = mybir.MatmulPerfMode.DoubleRow
    Copy = mybir.ActivationFunctionType.Copy
    Ident = mybir.ActivationFunctionType.Identity

    nc = bacc.Bacc(None, target_bir_lowering=False)
    x_ext = nc.declare_dram_parameter("x", [BPC, L, C], f32, isOutput=False)
    tok_ext = nc.declare_dram_parameter("pad_token", [1, C, 2], f32, isOutput=False)
    w_ext = nc.declare_dram_parameter("conv_w", [C, 1, 3, 3], f32, isOutput=False)
    b_ext = nc.declare_dram_parameter("conv_b", [C], f32, isOutput=False)
    out_ext = nc.declare_dram_parameter("out", [BPC, L, C], f32, isOutput=True)

    with TileContext(nc) as tc:
        with (
            tc.tile_pool(name="const", bufs=1) as constp,
            tc.tile_pool(name="xpool", bufs=54) as xpool,
            tc.tile_pool(name="zpool", bufs=8) as zpool,
            tc.tile_pool(name="qpool", bufs=3) as qpool,
            tc.tile_pool(name="opool", bufs=3) as opool,
            tc.tile_pool(name="ps_t", bufs=2, space="PSUM") as ps_t,
            tc.tile_pool(name="ps_z", bufs=2, space="PSUM") as ps_z,
            tc.tile_pool(name="ps_o", bufs=2, space="PSUM") as ps_o,
        ):
            # ---- constants ----
            ident = constp.tile([128, 128], bf16, tag="ident")
            make_identity(nc, ident)
            zeros = constp.tile([128, 128], bf16, tag="zeros")
            nc.vector.memset(zeros[:], 0.0)
            ident32 = constp.tile([128, 128], bf16, tag="ident32")
            nc.vector.tensor_scalar_mul(out=ident32[:], in0=ident[:], scalar1=32.0)
            identInv = constp.tile([128, 128], bf16, tag="identInv")
            nc.vector.tensor_scalar_mul(out=identInv[:], in0=ident[:], scalar1=1.0 / 32.0)

            wt = constp.tile([128, 9 * NCT], f32, tag="wt")
            cb = constp.tile([128, NCT], f32, tag="cb")
            cb32 = constp.tile([128, NCT], f32, tag="cb32")
            tokt = constp.tile([128, 2 * NCT], f32, tag="tokt")
            nc.sync.dma_start(
                out=wt.rearrange("p (ct t) -> p ct t", t=9),
                in_=w_ext.rearrange("(ct p) a k l -> p ct (a k l)", ct=NCT),
            )
            nc.sync.dma_start(
                out=cb[:],
                in_=b_ext.rearrange("(ct p) -> p ct", ct=NCT),
            )
            nc.sync.dma_start(
                out=tokt.rearrange("p (ct two) -> p ct two", two=2),
                in_=tok_ext.rearrange("a (ct p) two -> p ct (a two)", ct=NCT),
            )
            nc.vector.tensor_scalar_mul(out=cb32[:], in0=cb[:], scalar1=32.0)

            # ---- DoubleRow stationaries: 6 per ct, [128, 2, 128] fp8 ----
            # d[k, m, i] = delta(k,i) * 32 * w_m(k).  Pairings (m0, m1):
            #   dA: (-1,-1),(+1,-1)   dB: (-1,0),(+1,0)   dC: (-1,+1),(+1,+1)
            #   dD: (0,-1),(0,0)      dE: (0,+1), zero    dF: zero, (0,+1)
            def tcol(ct, di, dj):
                return ct * 9 + (di + 1) * 3 + (dj + 1)

            PAIRS = [
                ("dA", (-1, -1), (1, -1)),
                ("dB", (-1, 0), (1, 0)),
                ("dC", (-1, 1), (1, 1)),
                ("dD", (0, -1), (0, 0)),
                ("dE", (0, 1), None),
                ("dF", None, (0, 1)),
            ]
            dstat = {}  # (ct, name) -> tile
            for ct in range(NCT):
                for name, t0, t1 in PAIRS:
                    d = constp.tile([128, 256], fp8, tag=f"{name}{ct}")
                    dv = d.rearrange("p (m c) -> p m c", m=2)
                    for m, tt in ((0, t0), (1, t1)):
                        if tt is None:
                            nc.vector.memset(dv[:, m : m + 1, :], 0.0)
                        else:
                            nc.vector.tensor_scalar_mul(
                                out=dv[:, m : m + 1, :],
                                in0=ident32[:],
                                scalar1=wt[:, tcol(ct, *tt) : tcol(ct, *tt) + 1],
                            )
                    dstat[(ct, name)] = d

            # pads: V[r<64, 64] = tok0 (strided col writes); V[64, s] =
            # tok_{s%2} for s=0..64 (Q0 row 65, via the pair view)
            def emit_pads(ct, Qv, Qm):
                nc.vector.tensor_scalar_add(
                    out=Qv[:, 0:1, 1:65, 65:66],
                    in0=zeros[:, 0:64],
                    scalar1=tokt[:, 2 * ct : 2 * ct + 1],
                )
                r65h = 65 * RS // 2  # pair index of row 65 start (RS even)
                nc.vector.tensor_scalar_add(  # even s -> odd cols 1,3..65
                    out=Qm[:, 0:1, r65h : r65h + 33, 1:2],
                    in0=zeros[:, 0:33],
                    scalar1=tokt[:, 2 * ct : 2 * ct + 1],
                )
                nc.vector.tensor_scalar_add(  # odd s -> even cols 2,4..64
                    out=Qm[:, 0:1, r65h + 1 : r65h + 33, 0:1],
                    in0=zeros[:, 0:32],
                    scalar1=tokt[:, 2 * ct + 1 : 2 * ct + 2],
                )

            # ---- per-(bi,ct) phase A: build Q0/Q1/Q2 ----
            def emitA(bi, ct, xs, QQs):
                QQ = qpool.tile([128, 3 * QSZ], fp8, tag="QQ")
                QQs[ct] = QQ
                Qv = QQ.rearrange("p (v r c) -> p v r c", v=3, c=RS)
                Qm = QQ.rearrange("p (v m x) -> p v m x", v=3, x=2)
                # ring zeros and pad tokens first (no transpose dependency),
                # so the Q1/Q2 copies only wait on the last ACT evac
                nc.vector.memset(Qv[:, 0:1, 0:1, 0:66], 0.0)
                nc.vector.memset(Qv[:, 0:1, 1:66, 0:1], 0.0)
                emit_pads(ct, Qv, Qm)
                for g in range(8):  # 4 consecutive stripes per group
                    pst = ps_t.tile([128, 512], f32, tag="pst")
                    for k in range(4):
                        s = g * 4 + k
                        nc.tensor.matmul(
                            pst[:, k * 128 : (k + 1) * 128],
                            xs[s][:, ct * 128 : (ct + 1) * 128],
                            ident[:],
                            start=True,
                            stop=True,
                        )
                    # pst[c, il*64+j] = x(8g+il, j) -> Q0 rows 8g+1..8g+9,
                    # cols 1..65 (contiguous inner runs of 64).  3 of 8
                    # copies go on DVE to balance engine load.
                    if g in (1, 3, 5):
                        nc.vector.tensor_copy(
                            out=Qv[:, 0:1, 8 * g + 1 : 8 * g + 9, 1:65],
                            in_=pst.rearrange("p (il j) -> p il j", il=8),
                        )
                    else:
                        nc.scalar.activation(
                            out=Qv[:, 0:1, 8 * g + 1 : 8 * g + 9, 1:65],
                            in_=pst.rearrange("p (il j) -> p il j", il=8),
                            func=Copy,
                            scale=1.0,
                        )
                # shifted copies: Q1[k] = Q0[k+1], Q2[k] = Q0[k+2]
                nc.scalar.dma_start(
                    out=QQ[:, QSZ : 2 * QSZ - 1], in_=QQ[:, 1:QSZ]
                )
                nc.sync.dma_start(
                    out=QQ[:, 2 * QSZ : 3 * QSZ - 2], in_=QQ[:, 2:QSZ]
                )

            # ---- per-(bi,ct) phase B: conv -> u (natural y order) ----
            # chunk n computes y(r,s) for rows r=8n..8n+7 (s-inner); the 5 DR
            # matmuls accumulate in one PSUM bank.  rhs AP dims:
            # [[3*QSZ,128],[pairD,2],[RS,8],[1,64]]; base must be 4B-aligned
            # and pairD 16B-aligned, hence the Q0/Q1/Q2 variants.
            def emitB(bi, ct, QQs, z_tiles):
                QQ = QQs[ct]
                u = zpool.tile([128, L], fp8, tag="u")
                # u[c, r*64 + s] = y(r,s) + cb, r-major: the evac stays
                # contiguous.  (Every attempt to produce s-major u cost more
                # than phase C saves: DVE scatter-out 5x, PE out-AP scatter
                # ~2x on the DR matmuls, ACT scatter is free from SBUF but
                # 4x from PSUM.)
                z_tiles[ct] = u
                for blk in range(4):
                    pz0 = ps_z.tile([128, 512], f32, tag="pz")
                    pz1 = ps_z.tile([128, 512], f32, tag="pz")
                    pzs = [pz0, pz1]
                    for t in range(5):
                        for half in range(2):
                            n = 2 * blk + half
                            if t < 3:  # dA/dB/dC: vertical pairs in Qt
                                off = t * QSZ + (8 * n) * RS
                                pd = 2 * RS
                                dn = ("dA", "dB", "dC")[t]
                            elif t == 3:  # dD: (0,-1)@Q0 with (0,0)@Q1
                                off = (8 * n + 1) * RS
                                pd = QSZ
                                dn = "dD"
                            else:  # (0,+1)@Q2 + zero-weight dummy
                                if n < 7:
                                    off = 2 * QSZ + (8 * n + 1) * RS
                                    dn = "dE"
                                else:  # dummy rows would overflow: swap roles
                                    off = 2 * QSZ + (8 * n - 1) * RS
                                    dn = "dF"
                                pd = 2 * RS
                            rhs = AP(
                                QQ.tensor,
                                off,
                                [[3 * QSZ, 128], [pd, 2], [RS, 8], [1, 64]],
                            )
                            lhsT = AP(
                                dstat[(ct, dn)].tensor,
                                0,
                                [[256, 128], [128, 2], [1, 128]],
                            )
                            nc.tensor.matmul(
                                pzs[half][:],
                                lhsT,
                                rhs,
                                start=(t == 0),
                                stop=(t == 4),
                                perf_mode=DR,
                            )
                    for half in range(2):
                        n = 2 * blk + half
                        nc.vector.tensor_scalar_add(
                            out=u[:, n * 512 : (n + 1) * 512],
                            in0=pzs[half][:],
                            scalar1=cb32[:, ct : ct + 1],
                        )

            # ---- main loops ----
            # x loads: emit the next batch's first PRELOAD stripes right
            # after this batch's (the 44-slot ring has 12 spare slots), so
            # they stream during the DMA-idle A+B window instead of
            # competing with the out-stores inside phase C's window.
            PRELOAD = 22

            def load_stripe(bi, s):
                xt = xpool.tile([128, C], bf16, tag="x")
                # SWDGE dma casts f32 -> bf16; plain natural stripe
                nc.gpsimd.dma_start(
                    out=xt[:], in_=x_ext[bi, s * 128 : (s + 1) * 128, :]
                )
                return xt

            # ---- phase C: transpose back, residual, store (one stripe) ----
            # stripe s needs z[c, i*64+j] = u[c, j*64+i]/32 for i in
            # {2s, 2s+1}: two column-tiled matmuls per ct with
            # single-free-dim strided lhsT (stride 64; the weights AP
            # only admits one free dim) and (1/32)*I moving.
            def emitC(bi, s, xs, z_tiles):
                p2 = ps_o.tile([128, 1024], f32, tag="p2")
                for ct in range(NCT):
                    for il2 in range(2):
                        lhsT = AP(
                            z_tiles[ct].tensor,
                            2 * s + il2,
                            [[L, 128], [64, 64]],
                        )
                        nc.tensor.matmul(
                            p2[
                                il2 * 64 : (il2 + 1) * 64,
                                ct * 128 : (ct + 1) * 128,
                            ],
                            lhsT,
                            identInv[:],
                            start=True,
                            stop=True,
                        )
                ob = opool.tile([128, C], f32, tag="ob")
                nc.vector.tensor_add(out=ob[:], in0=p2[:], in1=xs[s][:])
                nc.sync.dma_start(
                    out=out_ext[bi, s * 128 : (s + 1) * 128, :], in_=ob[:]
                )

            # Schedule (BPC == 2): bi0's phase C window is DMA-bound (out
            # stores + bi1 x-in share HBM), so bi1's first 4 phase-A builds
            # are interleaved into it to hide their PE/ACT work; bi1's A/B
            # pipeline then resumes one-ct-ahead (B emitted before the A
            # that reuses its QQ ring slot, keeping WAR order acyclic).
            xs0 = [load_stripe(0, s) for s in range(NSTRIPE)]
            xs1 = [load_stripe(1, s) for s in range(PRELOAD)]
            QQ0, z0 = {}, {}
            for ci in range(NCT + 1):
                if ci < NCT:
                    emitA(0, ci, xs0, QQ0)
                if ci >= 1:
                    emitB(0, ci - 1, QQ0, z0)
            xs1 += [load_stripe(1, s) for s in range(PRELOAD, NSTRIPE)]
            QQ1, z1 = {}, {}
            APTS = {11: 0, 18: 1, 25: 2}  # stripe -> bi1 ct to prebuild
            for s in range(NSTRIPE):
                emitC(0, s, xs0, z0)
                if s in APTS:
                    emitA(1, APTS[s], xs1, QQ1)
            for ci in range(NCT):  # B0,A3,B1,A4,...,B4,A7,B5,B6,B7
                emitB(1, ci, QQ1, z1)
                if ci + 3 < NCT:
                    emitA(1, ci + 3, xs1, QQ1)
            for s in range(NSTRIPE):
                emitC(1, s, xs1, z1)


    nc.finalize()
    return nc


def _get_compiled():
    if "nc" not in _CACHE:
        _CACHE["nc"] = _build_nc()
    return _CACHE["nc"]


def _run(inputs, trace=False):
    from concourse.bass_utils import run_bass_kernel_spmd

    x = np.ascontiguousarray(np.asarray(inputs["x"], dtype=np.float32))
    pad_token = np.ascontiguousarray(np.asarray(inputs["pad_token"], dtype=np.float32))
    conv_w = np.ascontiguousarray(np.asarray(inputs["conv_w"], dtype=np.float32))
    conv_b = np.ascontiguousarray(np.asarray(inputs["conv_b"], dtype=np.float32))
    seq_idx = np.asarray(inputs["seq_idx"]).astype(np.int64)

    nc = _get_compiled()
    in_maps = []
    for k in range(NCORES):
        in_maps.append(
            {
                "x": x[k * BPC : (k + 1) * BPC],
                "pad_token": pad_token,
                "conv_w": conv_w,
                "conv_b": conv_b,
            }
        )
    res = run_bass_kernel_spmd(nc, in_maps, core_ids=list(range(NCORES)), trace=trace)
    out = np.concatenate([r["out"] for r in res.results], axis=0)

    # The device kernel hardcodes the reference's transpose permutation in its
    # access patterns. If the harness ever supplies a different seq_idx,
    # correct on host: out = x + y[:, seq_idx]  with y = (out_dev - x) at the
    # hardcoded permutation undone.
    exp = _expected_seq_idx()
    if not np.array_equal(seq_idx, exp):
        y = (out - x)[:, np.argsort(exp), :]
        out = x + y[:, seq_idx, :]

    return out, getattr(res, "exec_time_ns", None)


def kernel(**inputs) -> np.ndarray:
    out, _ = _run(inputs, trace=False)
    return out
